# revision 24
# baseline (speedup 1.0000x reference)
"""Batched Householder reflection: s_new[b] = s[b] - 2*(v[b]@s[b])/(v[b]@v[b]) * v[b].

Full inputs v, s: [512, 512] f32. Sharded batch-parallel across 8 NeuronCores
(64 rows per core). Per core: rows on SBUF partitions, K=512 on the free axis.
v and s shards are stacked host-side into one [2, 64, 512] DRAM tensor.

Raw Bass (no Tile, no Block): this walrus codegen allows only ONE inline
sync-wait per instruction, so all cross-engine sync is standalone wait_ge.
The load is split across the two HWDGE engines (SP loads v, ACT loads s) so
the two 128KB transfers overlap; the store is likewise split K-wise across
SP/ACT. ACT prewarms its Square activation table while the DMAs fly.
  dot = rowsum(v*s)   (DVE scalar_tensor_tensor with accum_out)
  nsq = rowsum(v*v)   (ACT Square activation with accum_out, runs in parallel)
  coef = -2*dot/nsq   (tiny per-partition ops)
  out = coef*v + s    (one fused DVE op, per-partition scalar broadcast)
"""

import numpy as np

B, K = 512, 512
N_CORES = 8
B_LOC = B // N_CORES  # 64 rows per core

_nc = None


def _build():
    import concourse.bass as bass
    from concourse import mybir

    nc = bass.Bass("TRN2", debug=False, num_devices=N_CORES, num_swdge_queues=2)
    f32 = mybir.dt.float32

    vs = nc.dram_tensor("vs", [2, B_LOC, K], f32, kind="ExternalInput").ap()
    out = nc.dram_tensor("out", [B_LOC, K], f32, kind="ExternalOutput").ap()

    vs_t = nc.alloc_sbuf_tensor("vs_t", [B_LOC, 2, K], f32).ap()
    o_t = nc.alloc_sbuf_tensor("o_t", [B_LOC, K], f32).ap()
    junk_vs = nc.alloc_sbuf_tensor("junk_vs", [B_LOC, K], f32).ap()
    junk_vv = nc.alloc_sbuf_tensor("junk_vv", [B_LOC, K], f32).ap()
    warm = nc.alloc_sbuf_tensor("warm", [B_LOC, 1], f32).ap()
    dot = nc.alloc_sbuf_tensor("dot", [B_LOC, 1], f32).ap()
    nsq = nc.alloc_sbuf_tensor("nsq", [B_LOC, 1], f32).ap()
    rcp = nc.alloc_sbuf_tensor("rcp", [B_LOC, 1], f32).ap()
    coef = nc.alloc_sbuf_tensor("coef", [B_LOC, 1], f32).ap()

    dma_in = nc.alloc_semaphore("dma_in")
    act_done = nc.alloc_semaphore("act_done")
    dve_sem = nc.alloc_semaphore("dve_sem")
    dve_done = nc.alloc_semaphore("dve_done")
    dma_out = nc.alloc_semaphore("dma_out")

    mult = mybir.AluOpType.mult
    add = mybir.AluOpType.add
    Square = mybir.ActivationFunctionType.Square

    sp, act, ve = nc.sync, nc.scalar, nc.vector
    v_t = vs_t[:, 0, :]
    s_t = vs_t[:, 1, :]
    zero64 = nc.const_aps.scalar_like(0.0, dot[:])

    # ---- loads ----
    # Each issue engine's dynamic DMA queue serializes its transfers at
    # ~28 GB/s, and each dma_start costs ~600ns of issue time on the engine.
    # So fan the 256KB input across FOUR streams: SP and ACT (one HWDGE queue
    # each) take the top row-halves, Pool (SWDGE) takes the bottom halves.
    pl = nc.gpsimd
    # Pool exits the init barrier later and issues its two chunks serially,
    # so it gets slightly smaller bottom chunks than SP/ACT's top chunks.
    HB = 35
    sp.dma_start(out=vs_t[:HB, 0, :], in_=vs[0, :HB, :]).then_inc(dma_in, 16)
    act.dma_start(out=vs_t[:HB, 1, :], in_=vs[1, :HB, :]).then_inc(dma_in, 16)
    pl.dma_start(out=vs_t[HB:, 0, :], in_=vs[0, HB:, :]).then_inc(dma_in, 16)
    pl.dma_start(out=vs_t[HB:, 1, :], in_=vs[1, HB:, :]).then_inc(dma_in, 16)

    # ACT: prewarm the Square table while the DMAs are in flight
    act.activation(out=warm[:], in_=zero64, func=Square)
    act.wait_ge(dma_in, 64)
    act.activation(out=junk_vv[:], in_=v_t, func=Square, accum_out=nsq[:]).then_inc(
        act_done, 1
    )

    # DVE chain
    ve.wait_ge(dma_in, 64)
    ve.scalar_tensor_tensor(
        out=junk_vs[:],
        in0=v_t,
        scalar=1.0,
        in1=s_t,
        op0=mult,
        op1=mult,
        accum_out=dot[:],
    ).then_inc(dve_sem, 1)
    ve.wait_ge(act_done, 1)
    ve.reciprocal(out=rcp[:], in_=nsq[:]).then_inc(dve_sem, 1)
    # DVE writes are not visible to the next DVE instruction without a
    # completion wait (in-order issue != in-order write visibility).
    ve.wait_ge(dve_sem, 2)
    ve.scalar_tensor_tensor(
        out=coef[:], in0=dot[:], scalar=-2.0, in1=rcp[:], op0=mult, op1=mult
    ).then_inc(dve_sem, 1)
    ve.wait_ge(dve_sem, 3)
    ve.scalar_tensor_tensor(
        out=o_t[:],
        in0=v_t,
        scalar=coef[:],
        in1=s_t,
        op0=mult,
        op1=add,
    ).then_inc(dve_done, 2)

    # ---- stores: three streams (SP / ACT / Pool) ----
    # ACT's store issue is measurably slower (activation-pipe drain before
    # descriptor gen), so it gets the smallest chunk.
    sp.wait_ge(dve_done, 2)
    sp.dma_start(out=out[0:24, :], in_=o_t[0:24, :]).then_inc(dma_out, 16)
    act.wait_ge(dve_done, 2)
    act.dma_start(out=out[24:42, :], in_=o_t[24:42, :]).then_inc(dma_out, 16)
    pl.wait_ge(dve_done, 2)
    pl.dma_start(out=out[42:64, :], in_=o_t[42:64, :]).then_inc(dma_out, 16)

    # SP resets semaphores for re-execution (PJRT reuses the loaded NEFF;
    # semaphores persist between executions). Sems whose waiters have
    # provably passed (everything up to dve_done) clear while the store
    # transfers drain; dve_done/dma_out clear after the final wait proves
    # Pool and ACT passed their dve_done waits too.
    sp.wait_ge(dve_done, 2)
    for sem in (dma_in, act_done, dve_sem):
        sp.sem_clear(sem)
    sp.wait_ge(dma_out, 48)
    sp.sem_clear(dve_done)
    sp.sem_clear(dma_out)

    return nc


def kernel(i=None, v=None, s=None, **_):
    global _nc
    from concourse.bass_utils import run_bass_kernel_spmd

    if _nc is None:
        _nc = _build()

    v = np.asarray(v, dtype=np.float32)
    s = np.asarray(s, dtype=np.float32)
    in_maps = [
        {
            "vs": np.ascontiguousarray(
                np.stack(
                    [v[c * B_LOC : (c + 1) * B_LOC], s[c * B_LOC : (c + 1) * B_LOC]]
                )
            )
        }
        for c in range(N_CORES)
    ]
    res = run_bass_kernel_spmd(_nc, in_maps, core_ids=list(range(N_CORES)))
    return np.concatenate([r["out"] for r in res.results], axis=0)


# revision 25
# speedup vs baseline: 1.2351x; 1.2351x over previous
"""Batched Householder reflection: s_new[b] = s[b] - 2*(v[b]@s[b])/(v[b]@v[b]) * v[b].

Full inputs v, s: [512, 512] f32. Sharded batch-parallel across 8 NeuronCores
(64 rows per core). Per core: rows on SBUF partitions, K=512 on the free axis.
v and s shards are stacked host-side into one [2, 64, 512] DRAM tensor.

Raw Bass (no Tile, no Block): this walrus codegen allows only ONE inline
sync-wait per instruction, so all cross-engine sync is standalone wait_ge.
The load is split across the two HWDGE engines (SP loads v, ACT loads s) so
the two 128KB transfers overlap; the store is likewise split K-wise across
SP/ACT. ACT prewarms its Square activation table while the DMAs fly.
  dot = rowsum(v*s)   (DVE scalar_tensor_tensor with accum_out)
  nsq = rowsum(v*v)   (ACT Square activation with accum_out, runs in parallel)
  coef = -2*dot/nsq   (tiny per-partition ops)
  out = coef*v + s    (one fused DVE op, per-partition scalar broadcast)
"""

import numpy as np

B, K = 512, 512
N_CORES = 8
B_LOC = B // N_CORES  # 64 rows per core

_nc = None


def _build():
    import concourse.bass as bass
    from concourse import mybir

    nc = bass.Bass("TRN2", debug=False, num_devices=N_CORES, num_swdge_queues=2)
    f32 = mybir.dt.float32

    vs = nc.dram_tensor("vs", [2, B_LOC, K], f32, kind="ExternalInput").ap()
    out = nc.dram_tensor("out", [B_LOC, K], f32, kind="ExternalOutput").ap()

    vs_t = nc.alloc_sbuf_tensor("vs_t", [B_LOC, 2, K], f32).ap()
    o_t = nc.alloc_sbuf_tensor("o_t", [B_LOC, K], f32).ap()
    junk_vs = nc.alloc_sbuf_tensor("junk_vs", [B_LOC, K], f32).ap()
    junk_vv = nc.alloc_sbuf_tensor("junk_vv", [B_LOC, K], f32).ap()
    warm = nc.alloc_sbuf_tensor("warm", [B_LOC, 1], f32).ap()
    dot = nc.alloc_sbuf_tensor("dot", [B_LOC, 1], f32).ap()
    nsq = nc.alloc_sbuf_tensor("nsq", [B_LOC, 1], f32).ap()
    rcp = nc.alloc_sbuf_tensor("rcp", [B_LOC, 1], f32).ap()
    coef = nc.alloc_sbuf_tensor("coef", [B_LOC, 1], f32).ap()

    dma_in = nc.alloc_semaphore("dma_in")
    act_done = nc.alloc_semaphore("act_done")
    dve_sem = nc.alloc_semaphore("dve_sem")
    dve_done = nc.alloc_semaphore("dve_done")
    dma_out = nc.alloc_semaphore("dma_out")

    mult = mybir.AluOpType.mult
    add = mybir.AluOpType.add
    Square = mybir.ActivationFunctionType.Square

    sp, act, ve = nc.sync, nc.scalar, nc.vector
    v_t = vs_t[:, 0, :]
    s_t = vs_t[:, 1, :]
    zero64 = nc.const_aps.scalar_like(0.0, dot[:])

    # ---- loads ----
    # Each issue engine's dynamic DMA queue serializes its transfers at
    # ~28 GB/s, and each dma_start costs ~600ns of issue time on the engine.
    # So fan the 256KB input across FOUR streams: SP and ACT (one HWDGE queue
    # each) take the top row-halves, Pool (SWDGE) takes the bottom halves.
    pl = nc.gpsimd
    HB = B_LOC // 2  # 32 rows
    sp.dma_start(out=vs_t[:HB, 0, :], in_=vs[0, :HB, :]).then_inc(dma_in, 16)
    act.dma_start(out=vs_t[:HB, 1, :], in_=vs[1, :HB, :]).then_inc(dma_in, 16)
    pl.dma_start(out=vs_t[HB:, 0, :], in_=vs[0, HB:, :]).then_inc(dma_in, 16)
    pl.dma_start(out=vs_t[HB:, 1, :], in_=vs[1, HB:, :]).then_inc(dma_in, 16)

    # ACT: prewarm the Square table while the DMAs are in flight
    act.activation(out=warm[:], in_=zero64, func=Square)
    act.wait_ge(dma_in, 64)
    act.activation(out=junk_vv[:], in_=v_t, func=Square, accum_out=nsq[:]).then_inc(
        act_done, 1
    )

    # DVE chain
    ve.wait_ge(dma_in, 64)
    ve.scalar_tensor_tensor(
        out=junk_vs[:],
        in0=v_t,
        scalar=1.0,
        in1=s_t,
        op0=mult,
        op1=mult,
        accum_out=dot[:],
    ).then_inc(dve_sem, 1)
    ve.wait_ge(act_done, 1)
    ve.reciprocal(out=rcp[:], in_=nsq[:]).then_inc(dve_sem, 1)
    # DVE writes are not visible to the next DVE instruction without a
    # completion wait (in-order issue != in-order write visibility).
    ve.wait_ge(dve_sem, 2)
    ve.scalar_tensor_tensor(
        out=coef[:], in0=dot[:], scalar=-2.0, in1=rcp[:], op0=mult, op1=mult
    ).then_inc(dve_sem, 1)
    ve.wait_ge(dve_sem, 3)
    ve.scalar_tensor_tensor(
        out=o_t[:],
        in0=v_t,
        scalar=coef[:],
        in1=s_t,
        op0=mult,
        op1=add,
    ).then_inc(dve_done, 2)

    # ---- stores: three streams (SP / ACT / Pool) ----
    # ACT's store issue is measurably slower (activation-pipe drain before
    # descriptor gen), so it gets the smallest chunk.
    sp.wait_ge(dve_done, 2)
    sp.dma_start(out=out[0:24, :], in_=o_t[0:24, :]).then_inc(dma_out, 16)
    act.wait_ge(dve_done, 2)
    act.dma_start(out=out[24:42, :], in_=o_t[24:42, :]).then_inc(dma_out, 16)
    pl.wait_ge(dve_done, 2)
    pl.dma_start(out=out[42:64, :], in_=o_t[42:64, :]).then_inc(dma_out, 16)

    # SP resets semaphores for re-execution (PJRT reuses the loaded NEFF;
    # semaphores persist between executions). Sems whose waiters have
    # provably passed (everything up to dve_done) clear while the store
    # transfers drain; dve_done/dma_out clear after the final wait proves
    # Pool and ACT passed their dve_done waits too.
    sp.wait_ge(dve_done, 2)
    for sem in (dma_in, act_done, dve_sem):
        sp.sem_clear(sem)
    sp.wait_ge(dma_out, 48)
    sp.sem_clear(dve_done)
    sp.sem_clear(dma_out)

    return nc


def kernel(i=None, v=None, s=None, **_):
    global _nc
    from concourse.bass_utils import run_bass_kernel_spmd

    if _nc is None:
        _nc = _build()

    v = np.asarray(v, dtype=np.float32)
    s = np.asarray(s, dtype=np.float32)
    in_maps = [
        {
            "vs": np.ascontiguousarray(
                np.stack(
                    [v[c * B_LOC : (c + 1) * B_LOC], s[c * B_LOC : (c + 1) * B_LOC]]
                )
            )
        }
        for c in range(N_CORES)
    ]
    res = run_bass_kernel_spmd(_nc, in_maps, core_ids=list(range(N_CORES)))
    return np.concatenate([r["out"] for r in res.results], axis=0)
